# revision 1
# baseline (speedup 1.0000x reference)
"""Multi-head attention TRN2 Bass kernel (B=2, S=4096, D=256, H=8).

Sharding: 8 cores; core c handles batch c//4, a 1024-row query slice.
Each core computes its full output slice (all heads); host concatenates.

Device algorithm (per core, q=1024 rows, k=4096 positions), float32r
matmuls throughout (4x faster than fp32 on the PE; ~4e-4 rel err):
  - Host supplies qT/kT/vT (D-major transposes) so the projections produce
    QT/KT with head-dim on partitions directly, plus an additive fp8 mask
    (-240 where masked) transposed to [k, q].
  - Heads are processed in 4 waves of 2 per q-chunk of 512. Per k-block:
    scores^T[k,q] accumulate in PSUM via a K=32 f32r matmul (row-strip
    tile_position per head) plus an identity-weight fp8 DoubleRow matmul
    that adds the mask at 0.5 cycles/row.
  - exp on ScalarE reads 2 PSUM banks per op (scale=1/sqrt(32) folded in),
    writes probsT (f32r) to SBUF.
  - attn^T accumulates over k-blocks in PSUM with V augmented by a ones
    column (row 32 = softmax denominator); normalize by the reciprocal
    denominator (DVE + gpsimd partition_broadcast), then w_out matmul.
"""

import math
import os
import sys
import time

import numpy as np

sys.path.insert(0, "/opt/trn_rl_repo")

import ml_dtypes  # noqa: E402

import concourse.bass as bass  # noqa: E402
import concourse.mybir as mybir  # noqa: E402
from concourse import bacc  # noqa: E402
from concourse.bass import ts  # noqa: E402
from concourse.bass_utils import run_bass_kernel_spmd  # noqa: E402
from concourse.tile import TileContext  # noqa: E402

B = 2
S = 4096
D = 256
H = 8
DH = 32
NCORES = 8
CORES_PER_B = 4
QS = S // CORES_PER_B  # 1024 query rows per core
QCHUNK = 512
NQC = QS // QCHUNK  # 2
KB = S // 128  # 32 k-blocks
SCALE = 1.0 / math.sqrt(DH)
MASK_VAL = -240.0

F32 = mybir.dt.float32
F32R = mybir.dt.float32r
BF16 = mybir.dt.bfloat16
FP8 = mybir.dt.float8e4

LAST_EXEC_NS = None


def build_nc(reps=1, skip=(), use_f32r=True):
    skip = set(skip)
    nc = bacc.Bacc(None)
    FR = F32R if use_f32r else F32

    def r(ap):
        return ap

    qT_d = nc.declare_dram_parameter("qT", [D, QS], FR, isOutput=False)
    kT_d = nc.declare_dram_parameter("kT", [D, S], FR, isOutput=False)
    vT_d = nc.declare_dram_parameter("vT", [D, S], FR, isOutput=False)
    maskT_d = nc.declare_dram_parameter("maskT", [S, QS], FP8, isOutput=False)
    wq_d = nc.declare_dram_parameter("wq", [D, D], FR, isOutput=False)
    wk_d = nc.declare_dram_parameter("wk", [D, D], FR, isOutput=False)
    wv_d = nc.declare_dram_parameter("wv", [D, D], FR, isOutput=False)
    wo_d = nc.declare_dram_parameter("wo", [D, D], FR, isOutput=False)
    id_d = nc.declare_dram_parameter("ident", [128, 2, 128], FP8, isOutput=False)
    out_d = nc.declare_dram_parameter("out", [QS, D], F32, isOutput=True)

    with TileContext(nc) as tc:
        with (
            tc.tile_pool(name="consts", bufs=1) as consts,
            tc.tile_pool(name="big", bufs=1) as big,
            tc.tile_pool(name="stream", bufs=6) as stream,
            tc.tile_pool(name="mpool", bufs=8) as mpool,
            tc.tile_pool(name="ppool", bufs=6) as ppool,
            tc.tile_pool(name="small", bufs=4) as small,
            tc.tile_pool(name="ostage", bufs=2) as ostage,
            tc.tile_pool(name="ps_sc", bufs=3, space="PSUM") as ps_sc,
            tc.tile_pool(name="ps_at", bufs=2, space="PSUM") as ps_at,
        ):
            # ---- constants ----
            ident = consts.tile([128, 2, 128], FP8)
            nc.sync.dma_start(out=ident, in_=id_d[:, :, :])
            wq_sb = consts.tile([128, 2, D], FR)
            wk_sb = consts.tile([128, 2, D], FR)
            wv_sb = consts.tile([128, 2, D], FR)
            wo_sb = consts.tile([128, 2, D], FR)
            for w_sb, w_d in ((wq_sb, wq_d), (wk_sb, wk_d), (wv_sb, wv_d), (wo_sb, wo_d)):
                nc.sync.dma_start(
                    out=w_sb, in_=w_d[:, :].rearrange("(c p) n -> p c n", p=128)
                )
            zeros = consts.tile([128, QCHUNK], BF16)
            nc.vector.memset(zeros, 0.0)
            ones_src = consts.tile([128, KB * H], F32)
            nc.vector.memset(ones_src, 1.0)

            for _rep in range(reps):
                # ---- persistent activations ----
                QT_sb = big.tile([128, 2, QS], FR, tag="qt", name="QT_sb")
                KT_sb = big.tile([128, 2, S], FR, tag="kt", name="KT_sb")
                V_sb = big.tile([128, KB, H, DH + 1], FR, tag="vs", name="V_sb")
                attnT_sb = big.tile([128, 2, QS], FR, tag="an", name="attnT_sb")
                nc.vector.tensor_copy(
                    V_sb[:, :, :, DH], ones_src.rearrange("p (a b) -> p a b", a=KB)
                )

                # ---- projections: QT = wq^T @ qT (per d-half), same for KT ----
                for src_d, dst, ncols in ((qT_d, QT_sb, QS), (kT_d, KT_sb, S)):
                    w_sb = wq_sb if dst is QT_sb else wk_sb
                    csz = 512
                    for c in range(ncols // csz):
                        t0 = stream.tile([128, csz], FR, tag="st")
                        t1 = stream.tile([128, csz], FR, tag="st")
                        nc.sync.dma_start(out=t0, in_=src_d[0:128, ts(c, csz)])
                        nc.sync.dma_start(out=t1, in_=src_d[128:256, ts(c, csz)])
                        for jj in range(csz // 512):
                            j = c * (csz // 512) + jj
                            for half in range(2):
                                ps = ps_sc.tile([128, 512], F32, tag="sc", name="psproj")
                                nc.tensor.matmul(
                                    ps, r(w_sb[:, 0, ts(half, 128)]), r(t0[:, ts(jj, 512)]), start=True, stop=False
                                )
                                nc.tensor.matmul(
                                    ps, r(w_sb[:, 1, ts(half, 128)]), r(t1[:, ts(jj, 512)]), start=False, stop=True
                                )
                                nc.vector.tensor_copy(dst[:, half, ts(j, 512)], ps)

                # ---- V projection: V[s, d] = sum_di vT[di, s] * wv[di, d] ----
                for c in range(S // 512):
                    v0 = stream.tile([128, 512], FR, tag="st")
                    v1 = stream.tile([128, 512], FR, tag="st")
                    nc.sync.dma_start(out=v0, in_=vT_d[0:128, ts(c, 512)])
                    nc.sync.dma_start(out=v1, in_=vT_d[128:256, ts(c, 512)])
                    for sb_i in range(4):
                        kb = c * 4 + sb_i
                        pv = ps_sc.tile([128, D], F32, tag="sc", name="psv")
                        nc.tensor.matmul(
                            pv, r(v0[:, ts(sb_i, 128)]), r(wv_sb[:, 0, :]), start=True, stop=False
                        )
                        nc.tensor.matmul(
                            pv, r(v1[:, ts(sb_i, 128)]), r(wv_sb[:, 1, :]), start=False, stop=True
                        )
                        nc.vector.tensor_copy(
                            V_sb[:, kb, :, 0:DH],
                            pv.rearrange("p (h d) -> p h d", h=H),
                        )

                # ---- main attention loops ----
                for qc in range(NQC):
                    for wave in range(4):
                        at_tiles = []
                        for j in range(2):
                            at = ps_at.tile([DH + 1, QCHUNK], F32, tag="at", name="at")
                            at_tiles.append(at)
                        for kb in range(KB):
                            mtile = mpool.tile([128, QCHUNK], FP8, tag="mt", name="mt")
                            nc.sync.dma_start(
                                out=mtile, in_=maskT_d[ts(kb, 128), ts(qc, QCHUNK)]
                            )
                            if "heads" in skip:
                                continue
                            for pr in range(1):
                                sc = ps_sc.tile(
                                    [128, 2, QCHUNK], F32, tag="sc", name="sc"
                                )
                                if "scores_mm" not in skip:
                                    for i in range(2):
                                        h = wave * 2 + i
                                        strip = (h % 4) * 32
                                        nc.tensor.matmul(
                                            sc[:, i, :],
                                            KT_sb[strip : strip + 32, h // 4, ts(kb, 128)],
                                            QT_sb[strip : strip + 32, h // 4, ts(qc, QCHUNK)],
                                            start=True,
                                            stop=False,
                                            tile_position=(strip, 0),
                                        )
                                if "mask_mm" not in skip:
                                    mrhs = bass.AP(
                                        tensor=mtile.tensor,
                                        offset=mtile.offset,
                                        ap=[mtile.ap[0], [0, 2], mtile.ap[1]],
                                    )
                                    for i in range(2):
                                        nc.tensor.matmul(
                                            sc[:, i, :],
                                            ident[:, :, :],
                                            mrhs,
                                            start=("scores_mm" in skip),
                                            stop=True,
                                            perf_mode=mybir.MatmulPerfMode.DoubleRow,
                                        )
                                pb = ppool.tile([128, 2, QCHUNK], FR, tag="pb", name="pb")
                                if "exp" not in skip:
                                    nc.scalar.activation(
                                        pb[:, :, :],
                                        sc[:, :, :],
                                        mybir.ActivationFunctionType.Exp,
                                        scale=SCALE,
                                    )
                                else:
                                    nc.vector.tensor_copy(pb[:, 0, 0:1], sc[:, 0, 0:1])
                                if "attn_mm" in skip:
                                    continue
                                for i in range(2):
                                    j = i
                                    h = wave * 2 + i
                                    nc.tensor.matmul(
                                        at_tiles[j][0 : DH + 1, :],
                                        V_sb[:, kb, h, :],
                                        pb[:, i, :],
                                        start=(kb == 0),
                                        stop=(kb == KB - 1),
                                    )

                        # normalize: attnT = attn_unnorm^T * (1/denom) broadcast
                        for j in range(2):
                            if "norm" in skip or "heads" in skip or "attn_mm" in skip:
                                continue
                            recip = small.tile([1, QCHUNK], F32, tag="rc", name="recip")
                            nc.vector.reciprocal(
                                recip, at_tiles[j][DH : DH + 1, :]
                            )
                            rb = small.tile([DH, QCHUNK], F32, tag="rb", name="rb")
                            nc.gpsimd.partition_broadcast(rb, recip)
                            h = wave * 2 + j
                            nc.vector.tensor_mul(
                                attnT_sb[ts(h % 4, DH), h // 4, ts(qc, QCHUNK)],
                                at_tiles[j][0:DH, :],
                                rb,
                            )

                    # output projection for this q-chunk
                    for qb in range(QCHUNK // 128):
                        qoff = qc * QCHUNK + qb * 128
                        po = ps_sc.tile([128, D], F32, tag="sc", name="po")
                        nc.tensor.matmul(
                            po,
                            r(attnT_sb[:, 0, qoff : qoff + 128]),
                            r(wo_sb[:, 0, :]),
                            start=True,
                            stop=False,
                        )
                        nc.tensor.matmul(
                            po,
                            r(attnT_sb[:, 1, qoff : qoff + 128]),
                            r(wo_sb[:, 1, :]),
                            start=False,
                            stop=True,
                        )
                        ot = ostage.tile([128, D], F32, tag="ot", name="ot")
                        nc.vector.tensor_copy(ot, po)
                        nc.sync.dma_start(out=out_d[qoff : qoff + 128, :], in_=ot)

    nc.finalize()
    return nc


_NC_CACHE = None


def _get_nc():
    global _NC_CACHE
    if _NC_CACHE is None:
        _NC_CACHE = build_nc()
    return _NC_CACHE


def _prep_in_maps(q, k, v, mask, wq, wk, wv, w_out):
    f32 = np.float32
    bf16 = ml_dtypes.bfloat16
    qT = np.ascontiguousarray(np.transpose(np.asarray(q, f32), (0, 2, 1)))
    kT = np.ascontiguousarray(np.transpose(np.asarray(k, f32), (0, 2, 1)))
    vT = np.ascontiguousarray(np.transpose(np.asarray(v, f32), (0, 2, 1)))
    maskT = np.transpose(np.asarray(mask, bool), (0, 2, 1))
    fp8 = ml_dtypes.float8_e4m3
    maskT_add = (maskT.astype(f32) * f32(MASK_VAL)).astype(fp8)
    ident = np.zeros((128, 2, 128), fp8)
    ident[:, 0, :] = np.eye(128, dtype=fp8)
    wq = np.asarray(wq, f32)
    wk = np.asarray(wk, f32)
    wv = np.asarray(wv, f32)
    wo = np.asarray(w_out, f32)

    in_maps = []
    for c in range(NCORES):
        b = c // CORES_PER_B
        qs = slice((c % CORES_PER_B) * QS, (c % CORES_PER_B + 1) * QS)
        in_maps.append(
            {
                "qT": np.ascontiguousarray(qT[b][:, qs]),
                "kT": kT[b],
                "vT": vT[b],
                "maskT": np.ascontiguousarray(maskT_add[b][:, qs]),
                "wq": wq,
                "wk": wk,
                "wv": wv,
                "wo": wo,
                "ident": ident,
            }
        )
    return in_maps


def kernel(q, k, v, mask, wq, wk, wv, w_out):
    global LAST_EXEC_NS
    nc = _get_nc()
    in_maps = _prep_in_maps(q, k, v, mask, wq, wk, wv, w_out)
    trace = bool(os.environ.get("KERNEL_TRACE"))
    try:
        res = run_bass_kernel_spmd(nc, in_maps, list(range(NCORES)), trace=trace)
    except Exception:
        # A wedged NeuronCore (NRT_EXEC_UNIT_UNRECOVERABLE) is usually
        # transient under axon; one retry after a reset request recovers it.
        os.environ["NEURON_RT_RESET_CORES"] = "1"
        time.sleep(2)
        res = run_bass_kernel_spmd(nc, in_maps, list(range(NCORES)), trace=trace)
    LAST_EXEC_NS = res.exec_time_ns
    out = np.empty((B, S, D), np.float32)
    for c in range(NCORES):
        b = c // CORES_PER_B
        qs = slice((c % CORES_PER_B) * QS, (c % CORES_PER_B + 1) * QS)
        out[b, qs] = res.results[c]["out"]
    return out

